# revision 32
# baseline (speedup 1.0000x reference)
"""Trainium2 Bass kernel for nn_FLD_83236466197026 (dense_transformer).

Strategy: data-parallel over batch B=64 across 8 cores (8 batches/core).

Algebraic restructuring (validated vs the reference, rel err ~1e-3):
  * scores = sinT.T @ As + t * c1 with As folded from W_k and q on the
    host; the per-j constant cancels in num/den (softmax-ratio
    invariance); max-subtraction skipped (|scores| < 4).
  * sin arguments computed on the PE as a K=2 outer-product matmul from
    a [2, B*L/2] t-table; the per-channel bias bs rides the Sin
    activation's per-partition bias port.
  * the affine t*c1 term is accumulated into the scores PSUM as a K=16
    matmul: stationary t16 (chunk view of t), moving a host-built
    block-diagonal c1bd.
  * maskb == [M, M] so x[..., D:] == 1 exactly: the ones rows of W_o
    fold into beff; only W_o's X-half (Wox) is used.
  * coeffs/C1 run per batch with beff/b1 added by ones-row matmuls (b1
    lands on the tau-ones row, exact since tau row0 == 1).
  * z = c0 + t*c1 + t^2*c2 folds into the first MLP layer (transposed
    MLP): h1 = relu((coeffs @ W1).T @ [1; t; t^2]) with a shared
    [3, B*T] tau-table.
  * output produced transposed [D, T] f16; host upcasts + transposes.

Scheduling: the TRN2 PE clock reaches 2.4 GHz only after ~3us of
gapless work, so emission order keeps the PE queue saturated: all
sin-arg matmuls first, then a software-pipelined steady state where
stage b runs attention(b) interleaved with coeffs(b-1) and the dense
MLP(b-1), the masked-value multiply rides the vector queue mid-MLP,
and X/M casting DMAs (gpsimd SWDGE) are dispatched two batches ahead.
Small constants ship as three packed DMAs.
"""

import sys

if "/opt/trn_rl_repo" not in sys.path:
    sys.path.insert(0, "/opt/trn_rl_repo")

import numpy as np

N_CORES = 8
B, L, T, D = 64, 2048, 1024, 128
E, H, P = 512, 8, 3
LAT, HID = 256, 512
NB = B // N_CORES       # batches per core
NS = E // H             # sin channels (64)
J = H * P               # flattened (head, poly) dim (24)
NCH = L // 128          # l-chunks per batch (16)

# packed-constant column map (f16): As2, eye24, onesP, Ws2, c1bd, beff, b1row
_C_AS, _C_EYE, _C_ONE, _C_WS, _C_C1, _C_BE, _C_B1, _C_END = (
    0, 48, 72, 84, 212, 596, 852, 1364)

_PROG_CACHE = {}


def _scol(i):
    # score-psum column of chunk i: matmul g covers chunks (g, g+8)
    return 48 * (i % 8) + 24 * (i // 8)


def _build_program(nb=NB, phase=9):
    import concourse.bacc as bacc
    import concourse.mybir as mybir
    from concourse.tile import TileContext

    dt = mybir.dt
    AF = mybir.ActivationFunctionType
    ALU = mybir.AluOpType
    f32, f16, f32r = dt.float32, dt.float16, dt.float32r

    nc = bacc.Bacc("TRN2", target_bir_lowering=False, debug=False,
                   num_devices=N_CORES)

    t_d = nc.dram_tensor("t", [nb, L], f32, kind="ExternalInput")
    X_d = nc.dram_tensor("X", [nb, L, D], f32, kind="ExternalInput")
    M_d = nc.dram_tensor("M", [nb, L, D], f32, kind="ExternalInput")
    y_d = nc.dram_tensor("y", [nb, T], f32, kind="ExternalInput")
    cst_d = nc.dram_tensor("cst", [128, _C_END], f16, kind="ExternalInput")
    f32p_d = nc.dram_tensor("f32p", [128, 6], f32, kind="ExternalInput")
    wpk_d = nc.dram_tensor("wpk", [128, 5632], f16, kind="ExternalInput")
    ones16_d = nc.dram_tensor("ones16", [2, nb * 1024], f16,
                              kind="ExternalInput")
    o_d = nc.dram_tensor("o", [nb, D, T], f16, kind="ExternalOutput")

    with TileContext(nc) as tc:
        with (
            tc.tile_pool(name="pconst", bufs=1) as pc,
            tc.tile_pool(name="psin", bufs=nb) as psin,
            tc.tile_pool(name="pw16", bufs=nb) as pw16,
            tc.tile_pool(name="pxm", bufs=3) as pxm,
            tc.tile_pool(name="psmall", bufs=2) as psm,
            tc.tile_pool(name="ph1", bufs=2) as ph1,
            tc.tile_pool(name="ph2", bufs=2) as ph2,
            tc.tile_pool(name="pout", bufs=2) as pout,
            tc.tile_pool(name="ps", bufs=1, space="PSUM") as pp,
        ):
            # ---- packed constants ----
            cst = pc.tile([128, _C_END], f16, tag="cst")
            nc.sync.dma_start(out=cst[:], in_=cst_d[:])
            f32p = pc.tile([128, 6], f32, tag="f32p")
            nc.sync.dma_start(out=f32p[:], in_=f32p_d[:])
            As_sb = cst[:, _C_AS:_C_AS + 48]
            eye_sb = cst[0:24, _C_EYE:_C_EYE + 24]
            onesP = cst[0:1, _C_ONE:_C_ONE + 12]
            beff_sb = cst[0:1, _C_BE:_C_BE + LAT]
            b1_sb = cst[0:1, _C_B1:_C_B1 + HID]
            b2_sb = f32p[:, 0:4]
            b3_sb = f32p[:, 4:5]
            bs_sb = f32p[:, 5:6]
            Ws2_sb = cst[0:2, _C_WS:_C_WS + 128]
            c1bd_sb = cst[0:16, _C_C1:_C_C1 + 384]

            # t tables: t16 via sync f32 + vector cast; T2 via gpsimd cast
            tst = pc.tile([16, nb * 128], f32, tag="tst")
            nc.sync.dma_start(
                out=tst[:].rearrange("p (b l) -> p b l", l=128),
                in_=t_d[:].rearrange("b (h g l) -> (h g) b l", h=2, g=8))
            t16 = pc.tile([16, nb * 128], f16, tag="t16")
            T2 = pc.tile([2, nb * 1024], f16, tag="T2")
            nc.gpsimd.dma_start(
                out=T2[0:1, :].rearrange("p (b l) -> p b l", b=nb),
                in_=t_d[:, 0:1024])
            nc.gpsimd.dma_start(
                out=T2[1:2, :].rearrange("p (b l) -> p b l", b=nb),
                in_=t_d[:, 1024:2048])

            # tau table
            TmAll = pc.tile([4, nb * T], f16, tag="TmAll")
            nc.sync.dma_start(out=TmAll[0:1, :], in_=ones16_d[0:1, :])
            nc.sync.dma_start(out=TmAll[3:4, :], in_=ones16_d[1:2, :])
            yst = pc.tile([nb, T], f32, tag="yst")
            nc.sync.dma_start(out=yst[:], in_=y_d[:])

            # weights pack
            wpk = pc.tile([128, 5632], f16, tag="wpk")
            nc.sync.dma_start(out=wpk[:], in_=wpk_d[:])
            Wox_sb = wpk[:, 0:2048]
            W1_sb = wpk[:, 2048:3072]
            W2_sb = wpk[:, 3072:5120]
            W3_sb = wpk[:, 5120:5632]

            # ---- batch loads (gpsimd SWDGE, casting) ----
            Vs, X16s = [None] * nb, [None] * nb

            def load_batch(b):
                # V pair-major: pair g holds chunks (g, g+8) at cols
                # [512g, 512g+512) as [MX_g | M_g | MX_g8 | M_g8], matching
                # the w16 column adjacency of the scores block-diagonal
                V = pxm.tile([128, NCH * 2 * D], f16, tag="V", name=f"V{b}")
                Vv = V[:].rearrange("p (g i2 c) -> p i2 g c", g=8, c=2 * D)
                nc.gpsimd.dma_start(
                    out=Vv[:, 0, :, D:2 * D],
                    in_=M_d[b, 0:1024].rearrange("(g p) d -> p g d", p=128))
                nc.gpsimd.dma_start(
                    out=Vv[:, 1, :, D:2 * D],
                    in_=M_d[b, 1024:2048].rearrange("(g p) d -> p g d",
                                                    p=128))
                X16 = pxm.tile([128, NCH * D], f16, tag="X16", name=f"X16{b}")
                nc.gpsimd.dma_start(
                    out=X16[:].rearrange("p (i d) -> p i d", d=D),
                    in_=X_d[b].rearrange("(i p) d -> p i d", p=128))
                Vs[b], X16s[b] = V, X16

            def vmul(b):
                Vv = Vs[b][:].rearrange("p (g i2 c) -> p i2 g c",
                                        g=8, c=2 * D)
                nc.vector.tensor_mul(
                    Vv[:, :, :, 0:D],
                    X16s[b][:].rearrange("p (i2 g d) -> p i2 g d",
                                         i2=2, d=D),
                    Vv[:, :, :, D:2 * D])

            load_batch(0)
            load_batch(1)
            nc.vector.tensor_copy(t16[:], tst[:])
            y16st = pc.tile([nb, T], f16, tag="y16st")
            nc.vector.tensor_copy(y16st[:], yst[:])
            nc.sync.dma_start(
                out=TmAll[1:2, :].rearrange("p (b t) -> p b t", b=nb),
                in_=y16st[:])
            y2st = pc.tile([nb, T], f16, tag="y2st")
            nc.vector.tensor_mul(y2st[:], yst[:], yst[:])
            nc.sync.dma_start(
                out=TmAll[2:3, :].rearrange("p (b t) -> p b t", b=nb),
                in_=y2st[:])

            # ---- sin args on PE + Sin activations (all before any Exp) ----
            sinT = []
            for b in range(nb):
                st = psin.tile([128, 1024], f16, tag="sinT", name=f"sinT{b}")
                for hh in range(2):
                    ps = pp.tile([128, 512], f32, tag="pss", bufs=2,
                                 name=f"sa{b}_{hh}")
                    nc.tensor.matmul(
                        ps[:], Ws2_sb,
                        T2[:, 1024 * b + 512 * hh:1024 * b + 512 * (hh + 1)],
                        start=True, stop=True)
                    nc.scalar.activation(st[:, 512 * hh:512 * (hh + 1)],
                                         ps[:], AF.Sin, bias=bs_sb)
                sinT.append(st)

            if phase == 0:
                for b in range(nb):
                    ob = pout.tile([128, T], f16, tag="o", name=f"od{b}")
                    nc.vector.tensor_copy(ob[:], sinT[b][:])
                    nc.sync.dma_start(out=o_d[b], in_=ob[:])

            # ---- steady-state stage: attention(b) + coeffs(b-1) + MLP(b-1)
            xTs = [None] * nb
            C1ops = [None] * nb
            w16s = [None] * nb

            def scores_exp(b):
                ps_s = pp.tile([128, 384], f32, tag="pss", bufs=2,
                               name=f"ps_s{b}")
                nc.tensor.matmul(ps_s[:], t16[:, 128 * b:128 * (b + 1)],
                                 c1bd_sb, start=True, stop=False,
                                 skip_group_check=True)
                for g in range(8):
                    nc.tensor.matmul(ps_s[:, 48 * g:48 * (g + 1)],
                                     sinT[b][:, 128 * g:128 * (g + 1)],
                                     As_sb, start=False, stop=True,
                                     skip_group_check=True)
                w = pw16.tile([128, 512], f16, tag="w16", name=f"w16_{b}")
                wv = w[:].rearrange("p (g jh jl) -> p g jh jl", g=8, jh=2,
                                    jl=32)[:, :, :, 0:24]
                nc.scalar.activation(
                    wv, ps_s[:].rearrange("p (g jh jl) -> p g jh jl",
                                          g=8, jh=2, jl=24), AF.Exp)
                w16s[b] = w
                if phase == 1:
                    ob = pout.tile([128, T], f16, tag="o", name=f"od{b}")
                    nc.vector.tensor_copy(ob[:, 0:384], w[:])
                    nc.sync.dma_start(out=o_d[b], in_=ob[:])

            def stage(b):
                a = b - 1          # coeffs/MLP batch
                nd = None
                if a >= 0:
                    ps_cf = pp.tile([3, LAT], f32, tag="small", bufs=3,
                                    name=f"cf{a}")
                    for h in range(8):
                        nc.tensor.matmul(ps_cf[:],
                                         xTs[a][:, 3 * h:3 * (h + 1)],
                                         Wox_sb[:, LAT * h:LAT * (h + 1)],
                                         start=(h == 0), stop=False)
                    nc.tensor.matmul(ps_cf[:], onesP[:, 0:3], beff_sb,
                                     start=False, stop=True)
                    cf16 = psm.tile([3, LAT], f16, tag="cf16", name=f"cfs{a}")
                    nc.vector.tensor_copy(cf16[:], ps_cf[:])
                if b < nb:
                    nd = pp.tile([56, 2 * 2 * D], f32, tag="small", bufs=3,
                                 name=f"nd{b}")
                    for g in range(4):
                        nc.tensor.matmul(nd[:],
                                         w16s[b][:, 64 * g:64 * g + 56],
                                         Vs[b][:, 512 * g:512 * (g + 1)],
                                         start=(g == 0), stop=False,
                                         skip_group_check=True)
                if a >= 0:
                    ctT = psm.tile([128, 8], f16, tag="ctT", name=f"ctT{a}")
                    nc.vector.memset(ctT[:], 0.0)
                    for k2 in range(2):
                        ps_ct = pp.tile([128, 3], f16, tag="small", bufs=3,
                                        name=f"ct{a}_{k2}")
                        nc.tensor.transpose(ps_ct[:],
                                            cf16[:, 128 * k2:128 * (k2 + 1)],
                                            eye_sb[0:3, 0:3])
                        nc.vector.tensor_copy(ctT[:, 4 * k2:4 * k2 + 3],
                                              ps_ct[:])
                    ps_c1 = pp.tile([4, HID], f32, tag="small", bufs=3,
                                    name=f"c1_{a}")
                    for k2 in range(2):
                        nc.tensor.matmul(ps_c1[:], ctT[:, 4 * k2:4 * (k2 + 1)],
                                         W1_sb[:, HID * k2:HID * (k2 + 1)],
                                         start=(k2 == 0), stop=False)
                    nc.tensor.matmul(ps_c1[:], onesP[:, 6:10], b1_sb,
                                     start=False, stop=True)
                    C1s = psm.tile([4, HID], f16, tag="C1s", name=f"C1s{a}")
                    for q in range(4):
                        nc.vector.tensor_copy(C1s[:, 128 * q:128 * (q + 1)],
                                              ps_c1[:, 128 * q:128 * (q + 1)])
                    C1ops[a] = C1s
                    if phase == 3:
                        ob = pout.tile([128, T], f16, tag="o", name=f"od{a}")
                        nc.vector.tensor_copy(ob[0:4, 0:HID], C1s[:])
                        nc.sync.dma_start(out=o_d[a], in_=ob[:])
                x16 = None
                if b < nb:
                    for g in range(4, 8):
                        nc.tensor.matmul(nd[:],
                                         w16s[b][:, 64 * g:64 * g + 56],
                                         Vs[b][:, 512 * g:512 * (g + 1)],
                                         start=False, stop=(g == 7),
                                         skip_group_check=True)
                    # combine the diagonal blocks: lo -> SBUF, then add the
                    # hi block (32-aligned PSUM read)
                    ndlo = psm.tile([24, 2 * D], f32, tag="ndlo",
                                    name=f"ndlo{b}")
                    nc.vector.tensor_copy(ndlo[:], nd[0:24, 0:2 * D])
                    nds = psm.tile([24, 2 * D], f32, tag="ndsum",
                                   name=f"nds{b}")
                    nc.vector.tensor_tensor(nds[:], ndlo[:],
                                            nd[32:56, 2 * D:4 * D], ALU.add)
                    rden = psm.tile([24, D], f32, tag="rden")
                    nc.vector.reciprocal(rden[:], nds[:, D:2 * D])
                    x16 = psm.tile([24, D], f16, tag="x16", name=f"x16_{b}")
                    nc.vector.tensor_mul(x16[:], nds[:, 0:D], rden[:])
                    if phase == 2:
                        ob = pout.tile([128, T], f16, tag="o", name=f"od{b}")
                        nc.vector.tensor_copy(ob[0:24, 0:2 * D], nds[:])
                        nc.sync.dma_start(out=o_d[b], in_=ob[:])
                if b + 2 < nb:
                    scores_exp(b + 2)

                def emit_xt():
                    ps_xt = pp.tile([128, 24], f16, tag="small", bufs=3,
                                    name=f"xt{b}")
                    nc.tensor.transpose(ps_xt[:], x16[:], eye_sb)
                    xT = psm.tile([128, 24], f16, tag="xT", name=f"xT{b}")
                    nc.vector.tensor_copy(xT[:], ps_xt[:])
                    xTs[b] = xT

                if b < nb and (a < 0 or phase < 9):
                    emit_xt()
                if b + 2 < nb:
                    load_batch(b + 2)

                if a >= 0 and phase >= 9:
                    # ---- MLP(a) ----
                    C1op = C1ops[a]
                    h1s = [ph1.tile([128, T], f16, tag=f"h1_{m}",
                                    name=f"h1_{a}_{m}") for m in range(4)]
                    for m in range(4):
                        for tg in range(2):
                            ps = pp.tile([128, 512], f32, tag="mlp", bufs=2,
                                         name=f"ph1_{a}_{m}_{tg}")
                            nc.tensor.matmul(
                                ps[:], C1op[:, 128 * m:128 * (m + 1)],
                                TmAll[:, T * a + 512 * tg:
                                      T * a + 512 * (tg + 1)],
                                start=True, stop=True)
                            dst = h1s[m][:, 512 * tg:512 * (tg + 1)]
                            if m < 2:
                                nc.scalar.activation(dst, ps[:], AF.Relu)
                            else:
                                nc.vector.tensor_scalar_max(dst, ps[:], 0.0)
                    if b + 1 < nb:
                        vmul(b + 1)
                    h2s = [ph2.tile([128, T], f16, tag=f"h2_{m}",
                                    name=f"h2_{a}_{m}") for m in range(4)]
                    for m in range(4):
                        pss = [pp.tile([128, 512], f32, tag="mlp", bufs=2,
                                       name=f"ph2_{a}_{m}_{tg}")
                               for tg in range(2)]
                        for k in range(4):
                            for tg in range(2):
                                nc.tensor.matmul(
                                    pss[tg][:],
                                    W2_sb[:, HID * k + 128 * m:
                                          HID * k + 128 * (m + 1)],
                                    h1s[k][:, 512 * tg:512 * (tg + 1)],
                                    start=(k == 0), stop=(k == 3))
                        for tg in range(2):
                            nc.scalar.activation(
                                h2s[m][:, 512 * tg:512 * (tg + 1)],
                                pss[tg][:], AF.Relu, bias=b2_sb[:, m:m + 1])
                        if m == 1 and b < nb:
                            emit_xt()   # attention(b) transpose rides here
                    o_sb = pout.tile([128, T], f16, tag="o", name=f"o{a}")
                    pso = [pp.tile([128, 512], f32, tag="mlp", bufs=2,
                                   name=f"po_{a}_{tg}") for tg in range(2)]
                    for k in range(4):
                        for tg in range(2):
                            nc.tensor.matmul(
                                pso[tg][:], W3_sb[:, D * k:D * (k + 1)],
                                h2s[k][:, 512 * tg:512 * (tg + 1)],
                                start=(k == 0), stop=(k == 3))
                    for tg in range(2):
                        nc.vector.tensor_scalar_add(
                            o_sb[:, 512 * tg:512 * (tg + 1)], pso[tg][:],
                            b3_sb)
                    nc.sync.dma_start(out=o_d[a], in_=o_sb[:])
                elif b + 1 < nb:
                    vmul(b + 1)

            if phase >= 2:
                scores_exp(0)
                scores_exp(1)
                vmul(0)
                for b in range(nb + 1):
                    stage(b)

    nc.compile()
    return nc


def _fold_params(inp):
    """Host-side parameter folding (float64 for exactness, cast at the end)."""
    f8 = np.float64
    q = inp["query"][0].astype(f8) @ inp["W_q"].astype(f8) + inp["b_q"].astype(f8)
    Wk = inp["W_k"].astype(f8)
    ek = E // H
    A = np.zeros((E, J))
    for h in range(H):
        cols = slice(h * ek, (h + 1) * ek)
        for p in range(P):
            A[:, h * P + p] = Wk[:, cols] @ q[p, cols]
    A /= np.sqrt(ek)
    sinm = (np.arange(E) % H) == 0
    ws = inp["w_te"].astype(f8)[sinm]
    bs = inp["b_te"].astype(f8)[sinm]
    As = A[sinm]
    c1 = inp["w_te"].astype(f8)[~sinm] @ A[~sinm]
    # NOTE: the per-j constant (b_te part + b_k part) cancels in num/den.
    Wo = inp["W_o"].astype(f8)
    Wox = np.zeros((H * D, LAT))
    beff = inp["b_o"].astype(f8).copy()
    for h in range(H):
        Wox[h * D:(h + 1) * D] = Wo[h * 2 * D:h * 2 * D + D]
        beff += Wo[h * 2 * D + D:(h + 1) * 2 * D].sum(axis=0)
    f16 = np.float16

    cst = np.zeros((128, _C_END))
    cst[0:NS, _C_AS:_C_AS + J] = As
    cst[NS:128, _C_AS + J:_C_AS + 2 * J] = As
    cst[0:24, _C_EYE:_C_EYE + 24] = np.eye(24)
    cst[0, _C_ONE:_C_ONE + 6] = 1.0
    cst[0, _C_ONE + 6:_C_ONE + 12:3] = 1.0
    cst[0, _C_WS:_C_WS + 64] = ws
    cst[1, _C_WS + 64:_C_WS + 128] = ws
    for h in range(2):
        for g in range(8):
            cst[8 * h + g, _C_C1 + 48 * g + 24 * h:
                _C_C1 + 48 * g + 24 * h + 24] = c1
    cst[0, _C_BE:_C_BE + LAT] = beff
    cst[0, _C_B1:_C_B1 + HID] = inp["b1"].astype(f8)

    f32p = np.zeros((128, 6), np.float32)
    f32p[:, 0:4] = inp["b2"].astype(np.float32).reshape(4, 128).T
    f32p[:, 4] = inp["b3"].astype(np.float32)
    f32p[0:64, 5] = bs
    f32p[64:128, 5] = bs

    def chunked(w, kc, n):
        return np.ascontiguousarray(
            w.reshape(kc, 128, n).transpose(1, 0, 2).reshape(128, kc * n))

    wpk = np.concatenate([
        chunked(Wox, 8, LAT),
        chunked(inp["W1"].astype(f8), 2, HID),
        chunked(inp["W2"].astype(f8), 4, HID),
        chunked(inp["W3"].astype(f8), 4, D),
    ], axis=1)

    return {
        "cst": cst.astype(f16),
        "f32p": f32p,
        "wpk": wpk.astype(f16),
        "ones16": np.concatenate([np.ones((1, NB * 1024), f16),
                                  np.zeros((1, NB * 1024), f16)]),
    }


def kernel(**inputs):
    from concourse.bass_utils import run_bass_kernel_spmd

    if "prog" not in _PROG_CACHE:
        _PROG_CACHE["prog"] = _build_program(
            phase=_PROG_CACHE.get("phase", 9))
    nc = _PROG_CACHE["prog"]

    inp = {k: np.asarray(v) for k, v in inputs.items()}
    params = _fold_params(inp)
    in_maps = []
    for c in range(N_CORES):
        sl = slice(NB * c, NB * (c + 1))
        m = {
            "t": np.ascontiguousarray(inp["timesteps"][sl].astype(np.float32)),
            "X": np.ascontiguousarray(inp["X"][sl].astype(np.float32)),
            "M": np.ascontiguousarray(inp["M"][sl].astype(np.float32)),
            "y": np.ascontiguousarray(inp["y_time_steps"][sl].astype(np.float32)),
        }
        m.update(params)
        in_maps.append(m)

    res = run_bass_kernel_spmd(nc, in_maps, list(range(N_CORES)),
                               **_PROG_CACHE.get("run_kwargs", {}))
    _PROG_CACHE["last_results"] = res
    out = np.empty((B, T, D), np.float32)
    for c in range(N_CORES):
        out[NB * c:NB * (c + 1)] = (
            res.results[c]["o"].astype(np.float32).transpose(0, 2, 1))
    return out
